# revision 9
# baseline (speedup 1.0000x reference)
"""Multi-head causal attention with RoPE on 8 Trainium2 NeuronCores.

Sharding: data-parallel over batch (B=2) x tensor-parallel over heads
(16 heads -> 4 groups of 4). Core c handles batch c//4, heads
[(c%4)*4, (c%4)*4+4). Each core computes a partial y = attn_out @ W_o
for its head group; the host sums the 4 partials per batch (the "W_o
all-reduce").

Device kernel (per core, all matmuls bf16, fp32 PSUM accumulation):
  - x^T built on-chip via PE transposes (contraction over E needs E on
    partitions).
  - Q^T/K^T/V^T projections in "T layout" (dims on partitions, seq on
    free): out = W_chunk.T @ x^T_chunk accumulated over 8 E-chunks.
  - RoPE: the within-head pair shuffle is folded into a host-side
    permutation of W_q/W_k columns so the rotation partner sits 16
    partitions away inside the same 32-partition quadrant; on device a
    single DVE stream_shuffle + cos/sin multiply-adds apply the
    rotation. Scores are permutation-invariant since Q and K use the
    same permutation.
  - scores^T[t, q] = K^T_tile.T @ Q^T (only t-blocks <= q-block:
    causal skip), exp on ACT (scale=1/32 folded in), causal mask on
    diagonal blocks, P^T @ [V | 1] accumulated in PSUM -> out^T plus
    softmax denominators in one matmul (ones column appended to V).
  - normalize with reciprocal + gpsimd partition_broadcast, then
    y = out_norm^T.T @ W_o chunks.
"""

import os
import sys
from contextlib import ExitStack

import numpy as np

for _p in ("/opt/trn_rl_repo",):
    if os.path.isdir(_p) and _p not in sys.path:
        sys.path.insert(0, _p)

import ml_dtypes  # noqa: E402

BF16 = ml_dtypes.bfloat16

B, S, E = 2, 2048, 1024
H, DH = 16, 64
NCORES = 8
HPC = H // 4          # 4 heads per core
DC = HPC * DH         # 256 head dims per core
ATTN_SCALE = 1.0 / 32.0  # 1/sqrt(E)
ROPE_BASE = 10000.0
P = 128
NSB = S // P          # 16 sequence blocks
NEC = E // P          # 8 E chunks
MB = DC // P          # 2 partition blocks of head dims

_PROG = None


def _perm64():
    """perm[j] = original head-dim index stored at permuted position j.

    Quadrant q of the permuted layout holds RoPE pairs i in
    [16q, 16q+16): even elements (2i) at slots 0-15, odd (2i+1) at
    slots 16-31. The rotation partner is then always +-16 partitions
    away within one 32-partition quadrant (stream_shuffle range).
    """
    j = np.arange(64)
    qd, r = j // 32, j % 32
    i = 16 * qd + (r % 16)
    return 2 * i + (r >= 16)


def _cos_sin_tiles():
    pl = np.arange(P) % 64
    qd, r = pl // 32, pl % 32
    i = 16 * qd + (r % 16)
    inv = ROPE_BASE ** (-(2.0 * i) / DH)
    ang = np.arange(S)[None, :] * inv[:, None]          # (128, S)
    sgn = np.where(r < 16, -1.0, 1.0)[:, None]
    return ang, sgn


def _build_program(debug=False):
    import concourse.bacc as bacc
    import concourse.tile as tile
    from concourse import masks, mybir

    f32 = mybir.dt.float32
    bf16 = mybir.dt.bfloat16
    AF = mybir.ActivationFunctionType

    nc = bacc.Bacc("TRN2", target_bir_lowering=False, debug=False)
    xb = nc.dram_tensor("xb", [S, E], bf16, kind="ExternalInput").ap()
    wq = nc.dram_tensor("wq", [E, DC], bf16, kind="ExternalInput").ap()
    wk = nc.dram_tensor("wk", [E, DC], bf16, kind="ExternalInput").ap()
    wv = nc.dram_tensor("wv", [E, DC], bf16, kind="ExternalInput").ap()
    wo = nc.dram_tensor("wo", [DC, E], bf16, kind="ExternalInput").ap()
    cosr = nc.dram_tensor("cosr", [P, S], bf16, kind="ExternalInput").ap()
    sinr = nc.dram_tensor("sinr", [P, S], bf16, kind="ExternalInput").ap()
    cmask = nc.dram_tensor("cmask", [P, P], bf16, kind="ExternalInput").ap()
    y = nc.dram_tensor("y", [S, E], f32, kind="ExternalOutput").ap()
    if debug:
        dbg = {
            "dxT": nc.dram_tensor("dxT", [P, NEC, S], mybir.dt.bfloat16,
                                  kind="ExternalOutput").ap(),
            "dqcT": nc.dram_tensor("dqcT", [P, MB, S], mybir.dt.bfloat16,
                                   kind="ExternalOutput").ap(),
            "dqT": nc.dram_tensor("dqT", [P, MB, S], mybir.dt.bfloat16,
                                  kind="ExternalOutput").ap(),
            "dkT": nc.dram_tensor("dkT", [P, MB, S], mybir.dt.bfloat16,
                                  kind="ExternalOutput").ap(),
            "dvn": nc.dram_tensor("dvn", [P, NSB, HPC, 65], mybir.dt.bfloat16,
                                  kind="ExternalOutput").ap(),
            "donrm": nc.dram_tensor("donrm", [P, MB, S], mybir.dt.bfloat16,
                                    kind="ExternalOutput").ap(),
            "dacc": nc.dram_tensor("dacc", [DH, HPC, S], f32,
                                   kind="ExternalOutput").ap(),
            "dden": nc.dram_tensor("dden", [1, HPC, S], f32,
                                   kind="ExternalOutput").ap(),
        }

    with ExitStack() as ctx:
        tc = ctx.enter_context(tile.TileContext(nc))
        consts = ctx.enter_context(tc.tile_pool(name="consts", bufs=1))
        persist = ctx.enter_context(tc.tile_pool(name="persist", bufs=1))

        ident = consts.tile([P, P], bf16, tag="ident")
        masks.make_identity(nc, ident[:])
        cos_t = consts.tile([P, S], bf16, tag="cos")
        nc.sync.dma_start(cos_t[:], cosr)
        sin_t = consts.tile([P, S], bf16, tag="sin")
        nc.sync.dma_start(sin_t[:], sinr)
        msk_t = consts.tile([P, P], bf16, tag="msk")
        nc.sync.dma_start(msk_t[:], cmask)
        wq_t = consts.tile([P, NEC, DC], bf16, tag="wq")
        nc.sync.dma_start(wq_t[:], wq.rearrange("(c p) m -> p c m", p=P))
        wk_t = consts.tile([P, NEC, DC], bf16, tag="wk")
        nc.sync.dma_start(wk_t[:], wk.rearrange("(c p) m -> p c m", p=P))
        wv_t = consts.tile([P, NEC, DC], bf16, tag="wv")
        nc.sync.dma_start(wv_t[:], wv.rearrange("(c p) m -> p c m", p=P))
        wo_t = consts.tile([P, MB, E], bf16, tag="wo")
        nc.sync.dma_start(wo_t[:], wo.rearrange("(c p) n -> p c n", p=P))

        xT = persist.tile([P, NEC, S], bf16, tag="xT")
        qcT = persist.tile([P, MB, S], bf16, tag="qcT")
        kcT = persist.tile([P, MB, S], bf16, tag="kcT")
        vT = persist.tile([P, MB, S], bf16, tag="vT")
        qT = persist.tile([P, MB, S], bf16, tag="qT")
        kT = persist.tile([P, MB, S], bf16, tag="kT")
        vn = persist.tile([P, NSB, HPC, 65], bf16, tag="vn")
        onrm = persist.tile([P, MB, S], bf16, tag="onrm")

        # ---- Phase A: x^T, projections, RoPE, V natural ----
        with ExitStack() as actx:
            xnat = actx.enter_context(tc.tile_pool(name="xnat", bufs=3))
            tp_ps = actx.enter_context(
                tc.tile_pool(name="tp_ps", bufs=3, space="PSUM")
            )
            pr_ps = actx.enter_context(
                tc.tile_pool(name="pr_ps", bufs=2, space="PSUM")
            )
            rtmp = actx.enter_context(tc.tile_pool(name="rtmp", bufs=2))

            for sb_i in range(NSB):
                xt = xnat.tile([P, E], bf16, tag="xt")
                nc.sync.dma_start(xt[:], xb[sb_i * P:(sb_i + 1) * P, :])
                for ec in range(NEC):
                    ps = tp_ps.tile([P, P], bf16, tag="tp")
                    nc.tensor.transpose(ps[:], xt[:, ec * P:(ec + 1) * P], ident[:])
                    eng = nc.vector if (ec % 2 == 0) else nc.scalar
                    if eng is nc.vector:
                        nc.vector.tensor_copy(
                            xT[:, ec, sb_i * P:(sb_i + 1) * P], ps[:]
                        )
                    else:
                        nc.scalar.copy(xT[:, ec, sb_i * P:(sb_i + 1) * P], ps[:])

            for wt, dst in ((wq_t, qcT), (wk_t, kcT), (wv_t, vT)):
                for mb in range(MB):
                    for half in range(2):
                        ps = pr_ps.tile([P, S // 2], f32, tag="proj")
                        for ec in range(NEC):
                            for qt in range(2):
                                c0 = half * 1024 + qt * 512
                                nc.tensor.matmul(
                                    ps[:, qt * 512:(qt + 1) * 512],
                                    lhsT=wt[:, ec, mb * P:(mb + 1) * P],
                                    rhs=xT[:, ec, c0:c0 + 512],
                                    start=(ec == 0),
                                    stop=(ec == NEC - 1),
                                )
                        nc.vector.tensor_copy(
                            dst[:, mb, half * 1024:(half + 1) * 1024], ps[:]
                        )

            shuf_mask = list(range(16, 32)) + list(range(16))
            for src, dst in ((qcT, qT), (kcT, kT)):
                for mb in range(MB):
                    sh = rtmp.tile([P, S], bf16, tag="shuf")
                    nc.vector.stream_shuffle(sh[:], src[:, mb, :], shuf_mask)
                    nc.vector.tensor_mul(sh[:], sh[:], sin_t[:])
                    nc.vector.tensor_mul(dst[:, mb, :], src[:, mb, :], cos_t[:])
                    nc.vector.tensor_add(dst[:, mb, :], dst[:, mb, :], sh[:])

            # V natural layout (t on partitions) + ones column per head
            nc.vector.memset(vn[:, :, :, 64:65], 1.0)
            for mb in range(MB):
                for sb_i in range(NSB):
                    ps = tp_ps.tile([P, P], bf16, tag="tp")
                    nc.tensor.transpose(
                        ps[:], vT[:, mb, sb_i * P:(sb_i + 1) * P], ident[:]
                    )
                    nc.vector.tensor_copy(
                        vn[:, sb_i, 2 * mb:2 * mb + 2, 0:64],
                        ps[:].rearrange("p (a b) -> p a b", a=2),
                    )

        # ---- Phase B: attention per head ----
        with ExitStack() as bctx:
            sc_ps = bctx.enter_context(
                tc.tile_pool(name="sc_ps", bufs=2, space="PSUM")
            )
            ac_ps = bctx.enter_context(
                tc.tile_pool(name="ac_ps", bufs=1, space="PSUM")
            )
            ptp = bctx.enter_context(tc.tile_pool(name="ptp", bufs=3))
            dn = bctx.enter_context(tc.tile_pool(name="dn", bufs=2))

            for h in range(HPC):
                mb, off = h // 2, (h % 2) * DH
                acc = ac_ps.tile([65, S], f32, tag="acc")
                for ti in range(NSB):
                    t0 = ti * P
                    c = t0
                    while c < S:
                        cb = (c // 1024) * 1024  # aligned tile base
                        c1 = min(S, cb + 1024)
                        sc = sc_ps.tile([P, 1024], f32, tag="sc")
                        p0 = c
                        while p0 < c1:
                            p1 = min(c1, (p0 // 512 + 1) * 512)
                            nc.tensor.matmul(
                                sc[:, p0 - cb:p1 - cb],
                                lhsT=kT[off:off + DH, mb, t0:t0 + P],
                                rhs=qT[off:off + DH, mb, p0:p1],
                            )
                            p0 = p1
                        pt = ptp.tile([P, 1024], bf16, tag="pt")
                        nc.scalar.activation(
                            pt[:, c - cb:c1 - cb],
                            sc[:, c - cb:c1 - cb],
                            AF.Exp,
                            scale=ATTN_SCALE,
                        )
                        if c == t0:
                            nc.vector.tensor_mul(
                                pt[:, t0 - cb:t0 - cb + P],
                                pt[:, t0 - cb:t0 - cb + P],
                                msk_t[:],
                            )
                        # one PV piece per PSUM bank: bank b of acc gets
                        # contributions from ti=0..(4b+3); start on the
                        # first (ti==0), stop on the last (the diagonal
                        # piece of ti==4b+3).
                        p0 = c
                        while p0 < c1:
                            bk = p0 // 512
                            p1 = min(c1, (bk + 1) * 512)
                            nc.tensor.matmul(
                                acc[:, p0:p1],
                                lhsT=vn[:, ti, h, :],
                                rhs=pt[:, p0 - cb:p1 - cb],
                                start=(ti == 0),
                                stop=(ti == 4 * bk + 3),
                            )
                            p0 = p1
                        c = c1
                if debug:
                    dac = dn.tile([DH, S], f32, tag="dac")
                    nc.vector.tensor_copy(dac[:], acc[0:DH, :])
                    nc.sync.dma_start(dbg["dacc"][:, h, :], dac[:])
                    dde = dn.tile([1, S], f32, tag="dde")
                    nc.vector.tensor_copy(dde[:], acc[64:65, :])
                    nc.sync.dma_start(dbg["dden"][:, h, :], dde[:])
                den = dn.tile([1, S], f32, tag="den")
                nc.vector.reciprocal(den[:], acc[64:65, :])
                denb = dn.tile([DH, S], f32, tag="denb")
                nc.gpsimd.partition_broadcast(denb[:], den[:])
                nc.vector.tensor_mul(
                    onrm[off:off + DH, mb, :], acc[0:DH, :], denb[:]
                )

        if debug:
            nc.sync.dma_start(dbg["dxT"], xT[:])
            nc.sync.dma_start(dbg["dqcT"], qcT[:])
            nc.sync.dma_start(dbg["dqT"], qT[:])
            nc.sync.dma_start(dbg["dkT"], kT[:])
            nc.sync.dma_start(dbg["dvn"], vn[:])
            nc.sync.dma_start(dbg["donrm"], onrm[:])

        # ---- Phase C: output projection ----
        with ExitStack() as cctx:
            y_ps = cctx.enter_context(
                tc.tile_pool(name="y_ps", bufs=2, space="PSUM")
            )
            yo = cctx.enter_context(tc.tile_pool(name="yo", bufs=3))
            for sb_i in range(NSB):
                yp = y_ps.tile([P, E], f32, tag="yp")
                for mb in range(MB):
                    for half in range(2):
                        nc.tensor.matmul(
                            yp[:, half * 512:(half + 1) * 512],
                            lhsT=onrm[:, mb, sb_i * P:(sb_i + 1) * P],
                            rhs=wo_t[:, mb, half * 512:(half + 1) * 512],
                            start=(mb == 0),
                            stop=(mb == MB - 1),
                        )
                ys = yo.tile([P, E], f32, tag="ys")
                nc.vector.tensor_copy(ys[:], yp[:])
                nc.sync.dma_start(y[sb_i * P:(sb_i + 1) * P, :], ys[:])

    nc.compile()
    return nc


def get_program():
    global _PROG
    if _PROG is None:
        _PROG = _build_program()
    return _PROG


def make_in_maps(x, W_q, W_k, W_v, W_o):
    perm = _perm64()
    idx_local = (np.arange(DC) // 64) * 64 + perm[np.arange(DC) % 64]
    ang, sgn = _cos_sin_tiles()
    cos_np = np.cos(ang).astype(BF16)
    sin_np = (sgn * np.sin(ang)).astype(BF16)
    # scores tile is (t, q): keep t <= q -> upper triangular incl. diagonal
    cmask_np = np.triu(np.ones((P, P))).astype(BF16)
    in_maps = []
    for c in range(NCORES):
        b, hg = c // 4, c % 4
        base = hg * DC
        in_maps.append(
            dict(
                xb=np.ascontiguousarray(x[b].astype(BF16)),
                wq=np.ascontiguousarray(W_q[:, base + idx_local].astype(BF16)),
                wk=np.ascontiguousarray(W_k[:, base + idx_local].astype(BF16)),
                wv=np.ascontiguousarray(W_v[:, base:base + DC].astype(BF16)),
                wo=np.ascontiguousarray(W_o[base:base + DC, :].astype(BF16)),
                cosr=cos_np,
                sinr=sin_np,
                cmask=cmask_np,
            )
        )
    return in_maps


def kernel(x, W_q, W_k, W_v, W_o, _trace=False, _trace_cores=None):
    from concourse.bass_utils import run_bass_kernel_spmd

    x = np.asarray(x, dtype=np.float32)
    W_q = np.asarray(W_q, dtype=np.float32)
    W_k = np.asarray(W_k, dtype=np.float32)
    W_v = np.asarray(W_v, dtype=np.float32)
    W_o = np.asarray(W_o, dtype=np.float32)

    nc = get_program()
    in_maps = make_in_maps(x, W_q, W_k, W_v, W_o)
    res = run_bass_kernel_spmd(
        nc,
        in_maps,
        list(range(NCORES)),
        trace=_trace,
        trace_cores=_trace_cores,
    )
    y = np.zeros((B, S, E), np.float32)
    for c in range(NCORES):
        y[c // 4] += res.results[c]["y"]
    if _trace:
        return y, res
    return y
